# revision 21
# baseline (speedup 1.0000x reference)
"""Multi-head causal attention (B=4, T=2048, DM=1024, H=16, dk=dv=64) on 8
Trainium2 NeuronCores.

Sharding: core c handles batch b = c//2 and head-group g = c%2 (8 heads).
Data-parallel over batch x tensor-parallel over heads; no cross-core comm.

Per-core bass/Tile kernel (all matmuls bf16, PSUM accumulation fp32):
  - host pre-lays-out x^T (d on partitions), Wq||Wk stacked per head, Wv
    packed across heads, and the causal mask tiles, all in bf16.
  - inputs stream in via a few large contiguous DMAs (the per-trigger issue
    cost on the Sync queue is ~650ns, so 8 triggers instead of 50).
  - projections: qT/kT = (Wq||Wk)^T-stationary matmuls vs x^T;
    v in natural [t, dv] layout via x^T-stationary matmuls vs packed Wv.
  - attention, flash-style over 512-wide t-chunks and 128-wide s-tiles:
      S^T[s,t] = kT_slice.T @ qT_chunk          (PE, K=64)
      P = exp(S * dk^-0.5)                       (ScalarE, scale folded in)
      diagonal tiles: P *= causal 0/1 mask       (VectorE)
      O_aug^T[65, t] += [v | 1]^T-stationary @ P (PE, K=128, fp32 accum)
    row 64 of O_aug^T collects the softmax denominators.
  - scheduling: exp paces attention (~1us/pair on ScalarE vs ~850ns of PE
    work), so the NEXT head-pair's QK-projection matmuls are interleaved
    into the current pair's attention stream to keep the PE saturated.
    Diagonal pairs go first within a chunk (mask-mul off the critical
    tail); the last pair runs its chunks largest-first so the kernel tail
    is the shortest dependency chain.
  - O_aug^T chunks are copied to SBUF (bf16) and DMAed out unnormalized;
    the host does the final divide + transpose (O(T*DV) work).
"""
import numpy as np
import ml_dtypes

_BF16 = ml_dtypes.bfloat16

B, T, DM = 4, 2048, 1024
H, DK, DV = 16, 64, 64
N_CORES = 8
HPC = 8          # heads per core
NDC = DM // 128  # 8 d-chunks
NTT = T // 128   # 16 t/s tiles of 128
NTC = T // 512   # 4 t-chunks of 512
N_WARMUP = 18    # PE warmup matmuls covering the input-DMA window

_cached = None   # (nc, run_bass_kernel_spmd)

# Set by a driver (e.g. test.py) to collect an NTFF profile; the exec time
# lands in LAST_EXEC_NS.
TRACE = False
LAST_EXEC_NS = None


def _build_program():
    global _cached
    if _cached is not None:
        return _cached
    import concourse.bacc as bacc
    import concourse.mybir as mybir
    from concourse import tile

    bf16 = mybir.dt.bfloat16
    f32 = mybir.dt.float32
    Exp = mybir.ActivationFunctionType.Exp

    nc = bacc.Bacc()
    xt = nc.declare_dram_parameter("xt", [128, NTC, NDC, 512], bf16, isOutput=False)
    wqk = nc.declare_dram_parameter("wqk", [128, HPC, NDC, 128], bf16, isOutput=False)
    wv = nc.declare_dram_parameter("wv", [128, NDC, 512], bf16, isOutput=False)
    msk = nc.declare_dram_parameter("msk", [128, 1280], bf16, isOutput=False)
    ot = nc.declare_dram_parameter("ot", [HPC, DV + 1, T], bf16, isOutput=True)

    with tile.TileContext(nc) as tc:
        with (
            tc.tile_pool(name="consts", bufs=1) as consts,
            tc.tile_pool(name="vpool", bufs=1) as vpool,
            tc.tile_pool(name="qk", bufs=4) as qkpool,
            tc.tile_pool(name="pt", bufs=4) as ptpool,
            tc.tile_pool(name="osb", bufs=4) as opool,
            tc.tile_pool(name="proj_ps", bufs=2, space="PSUM") as proj_ps,
            tc.tile_pool(name="s_ps", bufs=2, space="PSUM") as s_ps,
            tc.tile_pool(name="o_ps", bufs=2, space="PSUM") as o_ps,
        ):
            # Big contiguous input DMAs, ordered so V-projection (wv + xt
            # chunk 0) unblocks first, then head 0/1's projections.
            wv_sb = consts.tile([128, NDC, 512], bf16)
            msk_sb = consts.tile([128, 1280], bf16)
            xt_sb = consts.tile([128, NTC, NDC, 512], bf16)
            wqk_sb = consts.tile([128, HPC, NDC, 128], bf16)
            nc.sync.dma_start(wv_sb[:], wv[:])
            nc.sync.dma_start(xt_sb[:, 0], xt[:, 0])
            nc.sync.dma_start(xt_sb[:, 1], xt[:, 1])
            nc.sync.dma_start(wqk_sb[:, 0:2], wqk[:, 0:2])
            nc.sync.dma_start(xt_sb[:, 2], xt[:, 2])
            nc.sync.dma_start(xt_sb[:, 3], xt[:, 3])
            nc.sync.dma_start(wqk_sb[:, 2:HPC], wqk[:, 2:HPC])
            nc.sync.dma_start(msk_sb[:], msk[:])

            # HAM warmup: dependency-free matmuls on memset tiles run during
            # the initial DMA wait, so the PE clock gate is at 8/8 with no
            # gate-down before the real matmuls start.
            wu_w = consts.tile([128, 128], bf16)
            wu_x = consts.tile([128, 512], bf16)
            nc.vector.memset(wu_w[:], 0.0)
            nc.gpsimd.memset(wu_x[:], 0.0)
            for i in range(N_WARMUP):
                ps = proj_ps.tile([128, 512], f32, name="ps_wu", tag="ps_qk")
                nc.tensor.matmul(ps[:], lhsT=wu_w[:], rhs=wu_x[:],
                                 start=True, stop=True)
            # also pull the ~2.7us exp ACT_TABLE_LOAD into the DMA wait
            wu_e = consts.tile([128, 512], bf16)
            nc.scalar.activation(wu_e[:], wu_x[:], Exp)

            # V projection: v_sb[s, j, h, 0:64] = v values, v_sb[s, j, h, 64]
            # = 1.0 (softmax-denominator column).  Split by head group so the
            # groups feeding the later head pairs can be DEFERRED and woven
            # into earlier pairs' attention as real PE filler (exp on ScalarE
            # paces attention, leaving the PE ~20% idle there).
            v_sb = vpool.tile([128, NTT, HPC, DV + 1], bf16)
            nc.gpsimd.memset(v_sb[:, :, :, DV], 1.0)

            def vproj_unit(tt, h0, nh):
                # one 128-t-tile of V projection for heads h0:h0+nh
                def u():
                    w = nh * DV
                    ps = proj_ps.tile([128, 512], f32, name="ps_v", tag="ps_qk")
                    tch, uu = tt // 4, tt % 4
                    for dc in range(NDC):
                        nc.tensor.matmul(
                            ps[:, 0:w],
                            lhsT=xt_sb[:, tch, dc, 128 * uu:128 * (uu + 1)],
                            rhs=wv_sb[:, dc, DV * h0:DV * (h0 + nh)],
                            start=(dc == 0),
                            stop=(dc == NDC - 1),
                        )
                    nc.vector.tensor_copy(
                        v_sb[:, tt, h0:h0 + nh, 0:DV],
                        ps[:, 0:w].rearrange("p (h e) -> p h e", h=nh),
                    )
                return u



            qk_tiles = {}

            def make_qk_tiles(h):
                # qk1 = [q | k] on partitions [0:64 | 64:128];
                # qk2 = [k | q] (swapped halves).  Row-packed S matmuls need
                # weights and fmap at the SAME base partition, so even s-tiles
                # use (k,q) from partitions 0:64 and odd s-tiles use (k,q)
                # from partitions 64:128.
                qk_tiles[h] = (
                    qkpool.tile([128, T], bf16, name=f"qk1_{h}", tag="qk1"),
                    qkpool.tile([128, T], bf16, name=f"qk2_{h}", tag="qk2"),
                )

            def dummy_unit(w=384):
                # dependency-free matmul: keeps the PE clock gate at 8/8 and
                # bridges exp-paced stalls where no real filler work remains
                def u():
                    ps = proj_ps.tile([128, 512], f32, name="ps_dum", tag="ps_qk")
                    nc.tensor.matmul(ps[:, 0:w], lhsT=wu_w[:], rhs=wu_x[:, 0:w],
                                     start=True, stop=True)
                return u

            def proj_units(h, tps=(0, 1)):
                # QK projection for head h as a stream of small emit-units so
                # it can be woven into another pair's attention. Two t-chunks
                # per weight load: both proj psum banks accumulate with the
                # same stationary wqk chunk, so half the LDWEIGHTS.
                # NOTE: a group's units (8 u_mm + u_copy) hold proj_ps ring
                # slots across the group, so filler streams must emit whole
                # groups contiguously (attention units may interleave; other
                # ps_qk-tag allocations must not).
                qk1, qk2 = qk_tiles[h]
                for tp in tps:
                    st = {}

                    def u_mm(dc, tp=tp, st=st):
                        if dc == 0:
                            st['pss'] = [
                                proj_ps.tile([128, 512], f32,
                                             name=f"ps_qk{u}", tag="ps_qk")
                                for u in (0, 1)
                            ]
                        for u in (0, 1):
                            tch = 2 * tp + u
                            nc.tensor.matmul(
                                st['pss'][u][:],
                                lhsT=wqk_sb[:, h, dc, :],
                                rhs=xt_sb[:, tch, dc, :],
                                start=(dc == 0),
                                stop=(dc == NDC - 1),
                            )

                    for dc in range(NDC):
                        yield (lambda dc=dc, u_mm=u_mm: u_mm(dc))

                    def u_copy(tp=tp, st=st, qk1=qk1, qk2=qk2):
                        for u in (0, 1):
                            sl = slice(512 * (2 * tp + u), 512 * (2 * tp + u + 1))
                            nc.vector.tensor_copy(qk1[:, sl], st['pss'][u][:])
                            # swapped halves, cheap SBUF->SBUF bf16 copies
                            nc.vector.tensor_copy(qk2[0:64, sl], qk1[64:128, sl])
                            nc.vector.tensor_copy(qk2[64:128, sl], qk1[0:64, sl])

                    yield u_copy

            def attn_units(h, c):
                # Attention for head h, one 512-wide t-chunk, causal, as a
                # stream of (unit, filler_ok) emit-units, then a copy+DMA-out
                # unit.  Diagonal pairs first so the chunk's last PV does not
                # wait on the mask multiply.
                #
                # The S matmuls run in 64x128 row-tiled mode (the two s-tiles
                # of a pair execute CONCURRENTLY in PE row groups 0/64); PV
                # and projections run in 128x128 mode.  Switching tiling mode
                # drains the PE array (~100ns), so units are emitted as
                # [S(p) S(p+1)] [PV(p) PV(p+1)] stretches — half the mode
                # switches — and fillers are only allowed inside 128-mode
                # stretches (filler_ok=False after S units).
                qk1, qk2 = qk_tiles[h]
                csl = slice(512 * c, 512 * (c + 1))
                pairs = list(range(4 * c, 4 * c + 4, 2)) + list(range(0, 4 * c, 2))
                npairs = len(pairs)
                st = {}
                for idx, j0 in enumerate(pairs):
                    def u_s(j0=j0, idx=idx, st=st):
                        if idx == 0:
                            st['po'] = o_ps.tile([DV + 1, 512], f32,
                                                 name="po", tag="po")
                        pS = s_ps.tile([128, 1024], f32, name="pS", tag="pS")
                        pt = ptpool.tile([128, 1024], bf16, name="pt", tag="pt")
                        # Diagonal s-tiles (relative index r = j - 4c in 0..3)
                        # are fully masked below t-offset 128*r, so S / exp /
                        # PV only cover t in [128*r, 512).  The u=1 tile's
                        # output is COMPACTED to start at psum col 512 so the
                        # pair's live region [f0A : 1024-f0B] stays contiguous
                        # and a single exp op covers it.
                        rA = j0 - 4 * c
                        rB = rA + 1
                        f0A = max(0, 128 * rA)
                        f0B = max(0, 128 * rB)
                        st[('pt', idx)] = pt
                        st[('f', idx)] = (f0A, f0B)
                        nc.tensor.matmul(
                            pS[:, f0A:512],
                            lhsT=qk2[0:64, 128 * j0:128 * (j0 + 1)],
                            rhs=qk1[0:64, 512 * c + f0A:512 * (c + 1)],
                            start=True,
                            stop=True,
                            tile_position=(0, 0),
                        )
                        nc.tensor.matmul(
                            pS[:, 512:1024 - f0B],
                            lhsT=qk1[64:128, 128 * (j0 + 1):128 * (j0 + 2)],
                            rhs=qk2[64:128, 512 * c + f0B:512 * (c + 1)],
                            start=True,
                            stop=True,
                            tile_position=(64, 0),
                        )
                        nc.scalar.activation(
                            pt[:, f0A:1024 - f0B], pS[:, f0A:1024 - f0B],
                            Exp, scale=DK ** -0.5,
                        )
                        if rA >= 0:
                            # diagonal pair: one multiply with the pre-packed
                            # causal mask (d0 pair at mask cols 0:896, d1 pair
                            # at 896:1280, laid out to match the compacted pt)
                            m0 = 0 if rA == 0 else 896
                            mw = 896 if rA == 0 else 384
                            nc.vector.tensor_mul(
                                pt[:, f0A:1024 - f0B], pt[:, f0A:1024 - f0B],
                                msk_sb[:, m0:m0 + mw],
                            )

                    def u_pv(j0=j0, idx=idx, st=st):
                        f0A, f0B = st[('f', idx)]
                        pt, po = st[('pt', idx)], st['po']
                        for u, j, f0 in ((0, j0, f0A), (1, j0 + 1, f0B)):
                            nc.tensor.matmul(
                                po[:, f0:512],
                                lhsT=v_sb[:, j, h, :],
                                rhs=pt[:, 512 * u + (f0 if u == 0 else 0):
                                       512 * (u + 1) - (0 if u == 0 else f0)],
                                start=(idx == 0 and u == 0),
                                stop=(idx == npairs - 1 and u == 1),
                            )

                    st[('us', idx)] = u_s
                    st[('upv', idx)] = u_pv

                for idx0 in range(0, npairs, 2):
                    yield (st[('us', idx0)], False)
                    # after the last u_s of a stretch the 64->128 mode switch
                    # happens regardless, so a 128-mode filler here is free
                    yield (st[('us', idx0 + 1)], True)
                    yield (st[('upv', idx0)], True)
                    yield (st[('upv', idx0 + 1)], True)

                def u_out(st=st):
                    o_sb = opool.tile([DV + 1, 512], bf16, name="o_sb", tag="o_sb")
                    nc.vector.tensor_copy(o_sb[:], st['po'][:])
                    nc.sync.dma_start(ot[h, :, csl], o_sb[:])

                yield (u_out, True)

            # Upfront phase, during the input-DMA window: V projection for
            # heads 0-3, then pair 0's QK projections (tp0 before tp1 since
            # tp1 needs the last xt chunks).
            for tt in range(NTT):
                vproj_unit(tt, 0, 4)()
            make_qk_tiles(0)
            make_qk_tiles(1)
            for u in proj_units(0, (0,)):
                u()
            for u in proj_units(1, (0,)):
                u()
            for u in proj_units(0, (1,)):
                u()
            for u in proj_units(1, (1,)):
                u()

            def weave(attn_stream, fillers, front=0):
                # Emit attention units, interspersing filler units (128-mode
                # matmuls) only after filler_ok units so they join existing
                # 128-mode stretches instead of adding PE mode switches: 1:1
                # for the first `front` fillers (deadline-constrained work),
                # then uniformly over the remainder.
                pi = 0
                n_ok = max(1, sum(1 for _, ok in attn_stream if ok) - front)
                ratio = max(0.0, (len(fillers) - front)) / n_ok
                acc = 0.0
                for au, ok in attn_stream:
                    au()
                    if not ok:
                        continue
                    if pi < min(front, len(fillers)):
                        fillers[pi]()
                        pi += 1
                        continue
                    acc += ratio
                    while acc >= 1.0 and pi < len(fillers):
                        fillers[pi]()
                        pi += 1
                        acc -= 1.0
                while pi < len(fillers):
                    fillers[pi]()
                    pi += 1

            # Attention windows.  Chunk boundaries in the 88-unit per-pair
            # stream: c0 ends at 10, c1 at 28, c2 at 54, c3 at 88.  Fillers
            # are assigned per segment so deadline-constrained work (v tiles
            # and qk columns read by the NEXT chunk) lands in time while the
            # remainder spreads evenly to match the exp-pacing deficit.
            for hp in range(HPC // 2):
                hA, hB = 2 * hp, 2 * hp + 1
                attn_stream = []
                for c in range(NTC):
                    attn_stream += list(attn_units(hA, c))
                    attn_stream += list(attn_units(hB, c))
                if hp < HPC // 2 - 1:
                    make_qk_tiles(2 * hp + 2)
                    make_qk_tiles(2 * hp + 3)
                if hp == 0:
                    # next pair's projections + some anti-gate filler
                    segs = [(88, list(proj_units(2)) + list(proj_units(3))
                             + [dummy_unit() for _ in range(4)])]
                elif hp == 1:
                    # pair-2 projections + VB (heads 4,5) tiles for its c0
                    segs = [(88, list(proj_units(4)) + list(proj_units(5))
                             + [vproj_unit(j, 4, 2) for j in range(4)])]
                elif hp == 2:
                    # VB j4..7 by c1, j8..11 by c2, j12..15 by c3; pair-3's
                    # tp0 projections and VC j0..3 by the next window
                    segs = [
                        (10, [vproj_unit(j, 4, 2) for j in range(4, 8)]),
                        (28, [vproj_unit(j, 4, 2) for j in range(8, 12)]
                             + list(proj_units(6, (0,)))),
                        (54, [vproj_unit(j, 4, 2) for j in range(12, 16)]
                             + list(proj_units(7, (0,)))),
                        (88, [vproj_unit(j, 6, 2) for j in range(4)]
                             + [dummy_unit() for _ in range(4)]),
                    ]
                else:
                    # last pair self-carries: VC tiles for its own later
                    # chunks and its tp1 projections (c0/c1 only need tp0
                    # columns), with back-biased anti-gate dummies
                    tp1h7 = list(proj_units(7, (1,)))
                    segs = [
                        (10, [vproj_unit(j, 6, 2) for j in range(4, 8)]),
                        (28, [vproj_unit(j, 6, 2) for j in range(8, 12)]
                             + list(proj_units(6, (1,))) + tp1h7[:5]),
                        (54, tp1h7[5:]
                             + [vproj_unit(j, 6, 2) for j in range(12, 16)]
                             + [dummy_unit() for _ in range(2)]),
                        (88, [dummy_unit() for _ in range(10)]),
                    ]
                lo = 0
                for hi, fillers in segs:
                    weave(attn_stream[lo:hi], fillers)
                    lo = hi

    nc.finalize()
    from concourse.bass_utils import run_bass_kernel_spmd
    _cached = (nc, run_bass_kernel_spmd)
    return _cached


def _prep_core_inputs(x, Wq, Wk, Wv, core):
    b, g = core // 2, core % 2
    xb = x[b].astype(_BF16)                                  # [T, DM]
    xt = np.ascontiguousarray(
        xb.T.reshape(NDC, 128, NTC, 512).transpose(1, 2, 0, 3)  # [p,tch,dc,t]
    )
    wq = Wq[HPC * g:HPC * (g + 1)].astype(_BF16)             # [8, DM, 64]
    wk = Wk[HPC * g:HPC * (g + 1)].astype(_BF16)
    wv = Wv[HPC * g:HPC * (g + 1)].astype(_BF16)
    wqk = np.concatenate([wq, wk], axis=2)                   # [h, DM, 128]
    wqk = np.ascontiguousarray(
        wqk.reshape(HPC, NDC, 128, 128).transpose(2, 0, 1, 3)  # [p, h, dc, f]
    )
    wvp = np.ascontiguousarray(
        wv.reshape(HPC, NDC, 128, DV).transpose(2, 1, 0, 3).reshape(128, NDC, 512)
    )
    return {"xt": xt, "wqk": wqk, "wv": wvp, "msk": _mask()}


_mask_cache = None


def _mask():
    # Packed causal masks matching the compacted diagonal-pair layout:
    # cols 0:896  = d0 pair: [r0 tile t 0:512 | r1 tile t 128:512]
    # cols 896:1280 = d1 pair: [r2 tile t 256:512 | r3 tile t 384:512]
    global _mask_cache
    if _mask_cache is None:
        p = np.arange(128)[:, None]
        m = np.zeros((128, 1280), np.float32)
        c = np.arange(512)[None, :]
        m[:, 0:512] = p <= c
        c = np.arange(384)[None, :]
        m[:, 512:896] = p <= c
        c = np.arange(256)[None, :]
        m[:, 896:1152] = p <= c
        c = np.arange(128)[None, :]
        m[:, 1152:1280] = p <= c
        _mask_cache = m.astype(_BF16)
    return _mask_cache


def kernel(x, Wq, Wk, Wv):
    global LAST_EXEC_NS
    nc, run_spmd = _build_program()
    in_maps = [_prep_core_inputs(x, Wq, Wk, Wv, c) for c in range(N_CORES)]
    res = run_spmd(nc, in_maps, list(range(N_CORES)), trace=TRACE)
    global _LAST_RES
    _LAST_RES = res
    LAST_EXEC_NS = res.exec_time_ns

    out = np.empty((B, T, H * DV), np.float32)
    for c in range(N_CORES):
        b, g = c // 2, c % 2
        otc = res.results[c]["ot"].astype(np.float32)  # [8, 65, T]
        o = otc[:, :DV, :] / otc[:, DV:DV + 1, :]      # [h, dv, t]
        out[b, :, 512 * g:512 * (g + 1)] = (
            o.transpose(2, 0, 1).reshape(T, HPC * DV)
        )
    return out


# revision 25
# speedup vs baseline: 1.0202x; 1.0202x over previous
"""Multi-head causal attention (B=4, T=2048, DM=1024, H=16, dk=dv=64) on 8
Trainium2 NeuronCores.

Sharding: core c handles batch b = c//2 and head-group g = c%2 (8 heads).
Data-parallel over batch x tensor-parallel over heads; no cross-core comm.

Per-core bass/Tile kernel (all matmuls bf16, PSUM accumulation fp32):
  - host pre-lays-out x^T (d on partitions), Wq||Wk stacked per head, Wv
    packed across heads, and the causal mask tiles, all in bf16.
  - inputs stream in via a few large contiguous DMAs (the per-trigger issue
    cost on the Sync queue is ~650ns, so 8 triggers instead of 50).
  - projections: qT/kT = (Wq||Wk)^T-stationary matmuls vs x^T;
    v in natural [t, dv] layout via x^T-stationary matmuls vs packed Wv.
  - attention, flash-style over 512-wide t-chunks and 128-wide s-tiles:
      S^T[s,t] = kT_slice.T @ qT_chunk          (PE, K=64)
      P = exp(S * dk^-0.5)                       (ScalarE, scale folded in)
      diagonal tiles: P *= causal 0/1 mask       (VectorE)
      O_aug^T[65, t] += [v | 1]^T-stationary @ P (PE, K=128, fp32 accum)
    row 64 of O_aug^T collects the softmax denominators.
  - scheduling: exp paces attention (~1us/pair on ScalarE vs ~850ns of PE
    work), so the NEXT head-pair's QK-projection matmuls are interleaved
    into the current pair's attention stream to keep the PE saturated.
    Diagonal pairs go first within a chunk (mask-mul off the critical
    tail); the last pair runs its chunks largest-first so the kernel tail
    is the shortest dependency chain.
  - O_aug^T chunks are copied to SBUF (bf16) and DMAed out unnormalized;
    the host does the final divide + transpose (O(T*DV) work).
"""
import numpy as np
import ml_dtypes

_BF16 = ml_dtypes.bfloat16

B, T, DM = 4, 2048, 1024
H, DK, DV = 16, 64, 64
N_CORES = 8
HPC = 8          # heads per core
NDC = DM // 128  # 8 d-chunks
NTT = T // 128   # 16 t/s tiles of 128
NTC = T // 512   # 4 t-chunks of 512
N_WARMUP = 18    # PE warmup matmuls covering the input-DMA window

_cached = None   # (nc, run_bass_kernel_spmd)

# Set by a driver (e.g. test.py) to collect an NTFF profile; the exec time
# lands in LAST_EXEC_NS.
TRACE = False
LAST_EXEC_NS = None


def _build_program():
    global _cached
    if _cached is not None:
        return _cached
    import concourse.bacc as bacc
    import concourse.mybir as mybir
    from concourse import tile

    bf16 = mybir.dt.bfloat16
    f32 = mybir.dt.float32
    Exp = mybir.ActivationFunctionType.Exp

    nc = bacc.Bacc()
    xt = nc.declare_dram_parameter("xt", [128, NTC, NDC, 512], bf16, isOutput=False)
    wqk = nc.declare_dram_parameter("wqk", [128, HPC, NDC, 128], bf16, isOutput=False)
    wv = nc.declare_dram_parameter("wv", [128, NDC, 512], bf16, isOutput=False)
    msk = nc.declare_dram_parameter("msk", [128, 1280], bf16, isOutput=False)
    ot = nc.declare_dram_parameter("ot", [HPC, DV + 1, T], bf16, isOutput=True)

    with tile.TileContext(nc) as tc:
        with (
            tc.tile_pool(name="consts", bufs=1) as consts,
            tc.tile_pool(name="vpool", bufs=1) as vpool,
            tc.tile_pool(name="qk", bufs=4) as qkpool,
            tc.tile_pool(name="pt", bufs=4) as ptpool,
            tc.tile_pool(name="osb", bufs=4) as opool,
            tc.tile_pool(name="proj_ps", bufs=2, space="PSUM") as proj_ps,
            tc.tile_pool(name="s_ps", bufs=2, space="PSUM") as s_ps,
            tc.tile_pool(name="o_ps", bufs=2, space="PSUM") as o_ps,
        ):
            # Big contiguous input DMAs, ordered so V-projection (wv + xt
            # chunk 0) unblocks first, then head 0/1's projections.
            wv_sb = consts.tile([128, NDC, 512], bf16)
            msk_sb = consts.tile([128, 1280], bf16)
            xt_sb = consts.tile([128, NTC, NDC, 512], bf16)
            wqk_sb = consts.tile([128, HPC, NDC, 128], bf16)
            nc.sync.dma_start(wv_sb[:], wv[:])
            nc.sync.dma_start(xt_sb[:, 0], xt[:, 0])
            nc.sync.dma_start(wqk_sb[:, 0:2], wqk[:, 0:2])
            for tch in range(1, NTC):
                nc.sync.dma_start(xt_sb[:, tch], xt[:, tch])
            nc.sync.dma_start(wqk_sb[:, 2:HPC], wqk[:, 2:HPC])
            nc.sync.dma_start(msk_sb[:], msk[:])

            # HAM warmup: dependency-free matmuls on memset tiles run during
            # the initial DMA wait, so the PE clock gate is at 8/8 with no
            # gate-down before the real matmuls start.
            wu_w = consts.tile([128, 128], bf16)
            wu_x = consts.tile([128, 512], bf16)
            nc.vector.memset(wu_w[:], 0.0)
            nc.gpsimd.memset(wu_x[:], 0.0)
            for i in range(N_WARMUP):
                ps = proj_ps.tile([128, 512], f32, name="ps_wu", tag="ps_qk")
                nc.tensor.matmul(ps[:], lhsT=wu_w[:], rhs=wu_x[:],
                                 start=True, stop=True)
            # also pull the ~2.7us exp ACT_TABLE_LOAD into the DMA wait
            wu_e = consts.tile([128, 512], bf16)
            nc.scalar.activation(wu_e[:], wu_x[:], Exp)

            # V projection: v_sb[s, j, h, 0:64] = v values, v_sb[s, j, h, 64]
            # = 1.0 (softmax-denominator column).  Split by head group so the
            # groups feeding the later head pairs can be DEFERRED and woven
            # into earlier pairs' attention as real PE filler (exp on ScalarE
            # paces attention, leaving the PE ~20% idle there).
            v_sb = vpool.tile([128, NTT, HPC, DV + 1], bf16)
            nc.gpsimd.memset(v_sb[:, :, :, DV], 1.0)

            def vproj_unit(tt, h0, nh):
                # one 128-t-tile of V projection for heads h0:h0+nh
                def u():
                    w = nh * DV
                    ps = proj_ps.tile([128, 512], f32, name="ps_v", tag="ps_qk")
                    tch, uu = tt // 4, tt % 4
                    for dc in range(NDC):
                        nc.tensor.matmul(
                            ps[:, 0:w],
                            lhsT=xt_sb[:, tch, dc, 128 * uu:128 * (uu + 1)],
                            rhs=wv_sb[:, dc, DV * h0:DV * (h0 + nh)],
                            start=(dc == 0),
                            stop=(dc == NDC - 1),
                        )
                    nc.vector.tensor_copy(
                        v_sb[:, tt, h0:h0 + nh, 0:DV],
                        ps[:, 0:w].rearrange("p (h e) -> p h e", h=nh),
                    )
                return u



            qk_tiles = {}

            def make_qk_tiles(h):
                # qk1 = [q | k] on partitions [0:64 | 64:128];
                # qk2 = [k | q] (swapped halves).  Row-packed S matmuls need
                # weights and fmap at the SAME base partition, so even s-tiles
                # use (k,q) from partitions 0:64 and odd s-tiles use (k,q)
                # from partitions 64:128.
                qk_tiles[h] = (
                    qkpool.tile([128, T], bf16, name=f"qk1_{h}", tag="qk1"),
                    qkpool.tile([128, T], bf16, name=f"qk2_{h}", tag="qk2"),
                )

            def dummy_unit(w=384):
                # dependency-free matmul: keeps the PE clock gate at 8/8 and
                # bridges exp-paced stalls where no real filler work remains
                def u():
                    ps = proj_ps.tile([128, 512], f32, name="ps_dum", tag="ps_qk")
                    nc.tensor.matmul(ps[:, 0:w], lhsT=wu_w[:], rhs=wu_x[:, 0:w],
                                     start=True, stop=True)
                return u

            def proj_units(h, tps=(0, 1)):
                # QK projection for head h as a stream of small emit-units so
                # it can be woven into another pair's attention. Two t-chunks
                # per weight load: both proj psum banks accumulate with the
                # same stationary wqk chunk, so half the LDWEIGHTS.
                # NOTE: a group's units (8 u_mm + u_copy) hold proj_ps ring
                # slots across the group, so filler streams must emit whole
                # groups contiguously (attention units may interleave; other
                # ps_qk-tag allocations must not).
                qk1, qk2 = qk_tiles[h]
                for tp in tps:
                    st = {}

                    def u_mm(dc, tp=tp, st=st):
                        if dc == 0:
                            st['pss'] = [
                                proj_ps.tile([128, 512], f32,
                                             name=f"ps_qk{u}", tag="ps_qk")
                                for u in (0, 1)
                            ]
                        for u in (0, 1):
                            tch = 2 * tp + u
                            nc.tensor.matmul(
                                st['pss'][u][:],
                                lhsT=wqk_sb[:, h, dc, :],
                                rhs=xt_sb[:, tch, dc, :],
                                start=(dc == 0),
                                stop=(dc == NDC - 1),
                            )

                    for dc in range(NDC):
                        yield (lambda dc=dc, u_mm=u_mm: u_mm(dc))

                    def u_copy(tp=tp, st=st, qk1=qk1, qk2=qk2):
                        for u in (0, 1):
                            sl = slice(512 * (2 * tp + u), 512 * (2 * tp + u + 1))
                            nc.vector.tensor_copy(qk1[:, sl], st['pss'][u][:])
                            # swapped halves, cheap SBUF->SBUF bf16 copies
                            nc.vector.tensor_copy(qk2[0:64, sl], qk1[64:128, sl])
                            nc.vector.tensor_copy(qk2[64:128, sl], qk1[0:64, sl])

                    yield u_copy

            def attn_units(h, c):
                # Attention for head h, one 512-wide t-chunk, causal, as a
                # stream of (unit, filler_ok) emit-units, then a copy+DMA-out
                # unit.  Diagonal pairs first so the chunk's last PV does not
                # wait on the mask multiply.
                #
                # The S matmuls run in 64x128 row-tiled mode (the two s-tiles
                # of a pair execute CONCURRENTLY in PE row groups 0/64); PV
                # and projections run in 128x128 mode.  Switching tiling mode
                # drains the PE array (~100ns), so units are emitted as
                # [S(p) S(p+1)] [PV(p) PV(p+1)] stretches — half the mode
                # switches — and fillers are only allowed inside 128-mode
                # stretches (filler_ok=False after S units).
                qk1, qk2 = qk_tiles[h]
                csl = slice(512 * c, 512 * (c + 1))
                pairs = list(range(4 * c, 4 * c + 4, 2)) + list(range(0, 4 * c, 2))
                npairs = len(pairs)
                st = {}
                for idx, j0 in enumerate(pairs):
                    def u_s(j0=j0, idx=idx, st=st):
                        if idx == 0:
                            st['po'] = o_ps.tile([DV + 1, 512], f32,
                                                 name="po", tag="po")
                        pS = s_ps.tile([128, 1024], f32, name="pS", tag="pS")
                        pt = ptpool.tile([128, 1024], bf16, name="pt", tag="pt")
                        # Diagonal s-tiles (relative index r = j - 4c in 0..3)
                        # are fully masked below t-offset 128*r, so S / exp /
                        # PV only cover t in [128*r, 512).  The u=1 tile's
                        # output is COMPACTED to start at psum col 512 so the
                        # pair's live region [f0A : 1024-f0B] stays contiguous
                        # and a single exp op covers it.
                        rA = j0 - 4 * c
                        rB = rA + 1
                        f0A = max(0, 128 * rA)
                        f0B = max(0, 128 * rB)
                        st[('pt', idx)] = pt
                        st[('f', idx)] = (f0A, f0B)
                        nc.tensor.matmul(
                            pS[:, f0A:512],
                            lhsT=qk2[0:64, 128 * j0:128 * (j0 + 1)],
                            rhs=qk1[0:64, 512 * c + f0A:512 * (c + 1)],
                            start=True,
                            stop=True,
                            tile_position=(0, 0),
                        )
                        nc.tensor.matmul(
                            pS[:, 512:1024 - f0B],
                            lhsT=qk1[64:128, 128 * (j0 + 1):128 * (j0 + 2)],
                            rhs=qk2[64:128, 512 * c + f0B:512 * (c + 1)],
                            start=True,
                            stop=True,
                            tile_position=(64, 0),
                        )
                        nc.scalar.activation(
                            pt[:, f0A:1024 - f0B], pS[:, f0A:1024 - f0B],
                            Exp, scale=DK ** -0.5,
                        )
                        if rA >= 0:
                            # diagonal pair: one multiply with the pre-packed
                            # causal mask (d0 pair at mask cols 0:896, d1 pair
                            # at 896:1280, laid out to match the compacted pt)
                            m0 = 0 if rA == 0 else 896
                            mw = 896 if rA == 0 else 384
                            nc.vector.tensor_mul(
                                pt[:, f0A:1024 - f0B], pt[:, f0A:1024 - f0B],
                                msk_sb[:, m0:m0 + mw],
                            )

                    def u_pv(j0=j0, idx=idx, st=st):
                        f0A, f0B = st[('f', idx)]
                        pt, po = st[('pt', idx)], st['po']
                        for u, j, f0 in ((0, j0, f0A), (1, j0 + 1, f0B)):
                            nc.tensor.matmul(
                                po[:, f0:512],
                                lhsT=v_sb[:, j, h, :],
                                rhs=pt[:, 512 * u + (f0 if u == 0 else 0):
                                       512 * (u + 1) - (0 if u == 0 else f0)],
                                start=(idx == 0 and u == 0),
                                stop=(idx == npairs - 1 and u == 1),
                            )

                    st[('us', idx)] = u_s
                    st[('upv', idx)] = u_pv

                for idx0 in range(npairs):
                    yield (st[('us', idx0)], True)
                    yield (st[('upv', idx0)], True)

                def u_out(st=st):
                    o_sb = opool.tile([DV + 1, 512], bf16, name="o_sb", tag="o_sb")
                    nc.vector.tensor_copy(o_sb[:], st['po'][:])
                    nc.sync.dma_start(ot[h, :, csl], o_sb[:])

                yield (u_out, True)

            # Upfront phase, during the input-DMA window: V projection for
            # heads 0-3, then pair 0's QK projections (tp0 before tp1 since
            # tp1 needs the last xt chunks).
            for tt in range(NTT):
                vproj_unit(tt, 0, 4)()
            make_qk_tiles(0)
            make_qk_tiles(1)
            for u in proj_units(0):
                u()
            for u in proj_units(1):
                u()

            def weave(attn_stream, fillers, front=0):
                # Emit attention units, interspersing filler units (128-mode
                # matmuls) only after filler_ok units so they join existing
                # 128-mode stretches instead of adding PE mode switches: 1:1
                # for the first `front` fillers (deadline-constrained work),
                # then uniformly over the remainder.
                pi = 0
                n_ok = max(1, sum(1 for _, ok in attn_stream if ok) - front)
                ratio = max(0.0, (len(fillers) - front)) / n_ok
                acc = 0.0
                for au, ok in attn_stream:
                    au()
                    if not ok:
                        continue
                    if pi < min(front, len(fillers)):
                        fillers[pi]()
                        pi += 1
                        continue
                    acc += ratio
                    while acc >= 1.0 and pi < len(fillers):
                        fillers[pi]()
                        pi += 1
                        acc -= 1.0
                while pi < len(fillers):
                    fillers[pi]()
                    pi += 1

            # Attention windows.  Chunk boundaries in the 88-unit per-pair
            # stream: c0 ends at 10, c1 at 28, c2 at 54, c3 at 88.  Fillers
            # are assigned per segment so deadline-constrained work (v tiles
            # and qk columns read by the NEXT chunk) lands in time while the
            # remainder spreads evenly to match the exp-pacing deficit.
            for hp in range(HPC // 2):
                hA, hB = 2 * hp, 2 * hp + 1
                attn_stream = []
                for c in range(NTC):
                    attn_stream += list(attn_units(hA, c))
                    attn_stream += list(attn_units(hB, c))
                if hp < HPC // 2 - 1:
                    make_qk_tiles(2 * hp + 2)
                    make_qk_tiles(2 * hp + 3)
                if hp == 0:
                    # next pair's projections + some anti-gate filler
                    fillers = (list(proj_units(2)) + list(proj_units(3))
                               + [dummy_unit() for _ in range(6)])
                    front = 0
                elif hp == 1:
                    # pair-2 projections + VB (heads 4,5) tiles for its c0
                    fillers = (list(proj_units(4)) + list(proj_units(5))
                               + [vproj_unit(j, 4, 2) for j in range(4)])
                    front = 0
                elif hp == 2:
                    # VB j4..11 first (needed by this pair's c1/c2), then
                    # pair-3's tp0 projections, VB j12..15 (c3), VC j0..3
                    fillers = ([vproj_unit(j, 4, 2) for j in range(4, 12)]
                               + list(proj_units(6, (0,)))
                               + [vproj_unit(j, 4, 2) for j in range(12, 16)]
                               + list(proj_units(7, (0,)))
                               + [vproj_unit(j, 6, 2) for j in range(4)])
                    front = 8
                else:
                    # last pair self-carries: VC tiles for its own later
                    # chunks and its tp1 projections (c0/c1 only need tp0
                    # columns), then anti-gate dummies
                    fillers = ([vproj_unit(j, 6, 2) for j in range(4, 8)]
                               + list(proj_units(6, (1,)))
                               + [vproj_unit(j, 6, 2) for j in range(8, 12)]
                               + list(proj_units(7, (1,)))
                               + [vproj_unit(j, 6, 2) for j in range(12, 16)]
                               + [dummy_unit() for _ in range(6)])
                    front = 26
                weave(attn_stream, fillers, front)

    nc.finalize()
    from concourse.bass_utils import run_bass_kernel_spmd
    _cached = (nc, run_bass_kernel_spmd)
    return _cached


def _prep_core_inputs(x, Wq, Wk, Wv, core):
    b, g = core // 2, core % 2
    xb = x[b].astype(_BF16)                                  # [T, DM]
    xt = np.ascontiguousarray(
        xb.T.reshape(NDC, 128, NTC, 512).transpose(1, 2, 0, 3)  # [p,tch,dc,t]
    )
    wq = Wq[HPC * g:HPC * (g + 1)].astype(_BF16)             # [8, DM, 64]
    wk = Wk[HPC * g:HPC * (g + 1)].astype(_BF16)
    wv = Wv[HPC * g:HPC * (g + 1)].astype(_BF16)
    wqk = np.concatenate([wq, wk], axis=2)                   # [h, DM, 128]
    wqk = np.ascontiguousarray(
        wqk.reshape(HPC, NDC, 128, 128).transpose(2, 0, 1, 3)  # [p, h, dc, f]
    )
    wvp = np.ascontiguousarray(
        wv.reshape(HPC, NDC, 128, DV).transpose(2, 1, 0, 3).reshape(128, NDC, 512)
    )
    return {"xt": xt, "wqk": wqk, "wv": wvp, "msk": _mask()}


_mask_cache = None


def _mask():
    # Packed causal masks matching the compacted diagonal-pair layout:
    # cols 0:896  = d0 pair: [r0 tile t 0:512 | r1 tile t 128:512]
    # cols 896:1280 = d1 pair: [r2 tile t 256:512 | r3 tile t 384:512]
    global _mask_cache
    if _mask_cache is None:
        p = np.arange(128)[:, None]
        m = np.zeros((128, 1280), np.float32)
        c = np.arange(512)[None, :]
        m[:, 0:512] = p <= c
        c = np.arange(384)[None, :]
        m[:, 512:896] = p <= c
        c = np.arange(256)[None, :]
        m[:, 896:1152] = p <= c
        c = np.arange(128)[None, :]
        m[:, 1152:1280] = p <= c
        _mask_cache = m.astype(_BF16)
    return _mask_cache


def kernel(x, Wq, Wk, Wv):
    global LAST_EXEC_NS
    nc, run_spmd = _build_program()
    in_maps = [_prep_core_inputs(x, Wq, Wk, Wv, c) for c in range(N_CORES)]
    res = run_spmd(nc, in_maps, list(range(N_CORES)), trace=TRACE)
    global _LAST_RES
    _LAST_RES = res
    LAST_EXEC_NS = res.exec_time_ns

    out = np.empty((B, T, H * DV), np.float32)
    for c in range(N_CORES):
        b, g = c // 2, c % 2
        otc = res.results[c]["ot"].astype(np.float32)  # [8, 65, T]
        o = otc[:, :DV, :] / otc[:, DV:DV + 1, :]      # [h, dv, t]
        out[b, :, 512 * g:512 * (g + 1)] = (
            o.transpose(2, 0, 1).reshape(T, HPC * DV)
        )
    return out
